# revision 11
# baseline (speedup 1.0000x reference)
"""Multi-head causal attention (B=4, S=2048, E=1024, H=16, Dh=64) on 8 TRN2
NeuronCores.

Sharding: core c -> batch b = c//2, head group hb = c%2 (8 heads each).
Each core computes Q/K/V projections for its 8 heads, causal softmax
attention, and a partial output projection over its 512 of the 1024
concat-head dims; a pairwise chunked ReduceScatter(add) sums the two head
groups, so core c returns 4 chunks of 256 sequence rows.

Layouts on device (transposed so no on-chip transposes are needed):
  xT   [E=1024, S=2048]  (host-pretransposed x[b])
  QT   [128, S] per head: rows po..po+63 = Q_h^T, other 64 rows zero so
       every scores matmul contracts over K=128 (the PE pays ~2x when the
       contraction dim alternates between 64 and 128 back-to-back)
  KT   [128 (2 heads x 64d), S] x 4 tiles
  V    [s-tile 128, 8 heads x 65] (65th col = ones -> softmax denominator)
  scoresT [k-tile 128, q 512] = KT_pair.T @ QT_pad; exp on ACT
  ctxT [65, q 512] accum over k-tiles = V_aug.T @ attnT  (row 64 = denom)
  out  [s-tile 128, e 512] accum over 4 f-tiles = ctxT.T @ Wo

Matmul operands are float32r (reduced-precision fp32 matmul, ~2e-4 rel
err, 4x the fp32 matmul rate). Softmax runs unnormalized (scores are
bounded, exp never overflows); each head's context rows are divided by
the ones-column denominator one block later, off the PE critical path
(recip broadcast via gpsimd partition_broadcast, not the PE).
The loop runs q-blocks outermost so the output projection and the
pairwise reduce-scatter of each finished q-chunk overlap the attention
of the next q-block.
"""

import numpy as np

B, S, E = 4, 2048, 1024
H, Dh = 16, 64
HL = 8          # heads per core
N_CORES = 8
SC = 0.125      # 1/sqrt(Dh)

_CACHE = {}


def _build():
    import concourse.bacc as bacc
    import concourse.mybir as mybir
    import concourse.tile as tile

    F32 = mybir.dt.float32
    F32R = mybir.dt.float32r
    Exp = mybir.ActivationFunctionType.Exp
    mult = mybir.AluOpType.mult
    add = mybir.AluOpType.add

    nc = bacc.Bacc("TRN2", target_bir_lowering=False, debug=False)

    xT = nc.dram_tensor("xT", [E, S], F32R, kind="ExternalInput")
    wq = nc.dram_tensor("wq", [E, 512], F32R, kind="ExternalInput")
    wk = nc.dram_tensor("wk", [E, 512], F32R, kind="ExternalInput")
    wv = nc.dram_tensor("wv", [E, 512], F32R, kind="ExternalInput")
    wo = nc.dram_tensor("wo", [512, E], F32R, kind="ExternalInput")
    bqk = nc.dram_tensor("bqk", [128, 8], F32, kind="ExternalInput")
    bvb = nc.dram_tensor("bvb", [128, 512], F32, kind="ExternalInput")
    bob = nc.dram_tensor("bob", [128, E], F32, kind="ExternalInput")
    maskt = nc.dram_tensor("maskt", [128, 128], F32R, kind="ExternalInput")
    onescol = nc.dram_tensor("onescol", [128, 8], F32R, kind="ExternalInput")
    out = nc.dram_tensor("out", [S // 2, E], F32, kind="ExternalOutput")

    NQT = S // 512          # 4 q-blocks of 512
    NST = S // 128          # 16 s-tiles of 128
    NET = E // 128          # 8 e-tiles

    with tile.TileContext(nc) as tc:
        with (
            tc.tile_pool(name="persist", bufs=1) as pp,
            tc.tile_pool(name="psum", bufs=1, space="PSUM") as psp,
            tc.tile_pool(name="dram", bufs=1, space="DRAM") as dram,
        ):
            # ---- persistent tiles ----
            qt8 = [pp.tile([128, S], F32R, tag="qt", bufs=8, name=f"qt{i}")
                   for i in range(HL)]
            kt_t = [pp.tile([128, S], F32R, tag="kt", bufs=4, name=f"kt{i}")
                    for i in range(4)]
            v_t = [pp.tile([128, HL * 65], F32R, tag="v", bufs=NST,
                           name=f"v{i}") for i in range(NST)]

            bqk_sb = pp.tile([128, 8], F32, tag="bqk")
            bvb_sb = pp.tile([128, 512], F32, tag="bvb")

            # ---- phase 1: projections (weights fully resident) ----
            with tc.tile_pool(name="ph1", bufs=1) as p1:
                wt = {0: [], 1: [], 2: []}
                for qt in range(4):           # s-quarters of 512
                    s0 = qt * 512
                    xts = []
                    for e in range(NET):
                        t = p1.tile([128, 512], F32R, tag="xt", bufs=12,
                                    name=f"xt{qt}_{e}")
                        nc.sync.dma_start(
                            t[:], xT.ap()[e * 128:(e + 1) * 128, s0:s0 + 512])
                        xts.append(t)
                        if qt == 0:
                            for wi, wd in [(0, wq), (1, wk), (2, wv)]:
                                w = p1.tile([128, 512], F32R, tag="w", bufs=24,
                                            name=f"w{wi}_{e}")
                                nc.sync.dma_start(
                                    w[:], wd.ap()[e * 128:(e + 1) * 128, :])
                                wt[wi].append(w)
                    if qt == 0:
                        nc.sync.dma_start(bqk_sb[:], bqk.ap())
                        nc.sync.dma_start(bvb_sb[:], bvb.ap())
                        # zero the pad rows of the per-head QT tiles
                        # (x*0; memset can't write float32r)
                        src = xts[0]
                        for h in range(HL):
                            pad = 64 - 64 * (h & 1)   # the *other* 64 rows
                            for half in range(4):
                                with nc.allow_low_precision(reason="f32r"):
                                    nc.vector.tensor_scalar_mul(
                                        qt8[h][pad:pad + 64,
                                               half * 512:(half + 1) * 512],
                                        src[0:64, :], 0.0)

                    cols = slice(s0, s0 + 512)
                    for wi in range(3):
                        ws = wt[wi]
                        if wi == 0:           # QT -> per-head padded tiles
                            for t4 in range(4):
                                ps = psp.tile([128, 512], F32, tag="acc",
                                              bufs=2, name="accp")
                                for e in range(NET):
                                    nc.tensor.matmul(
                                        ps[:],
                                        ws[e][:, t4 * 128:(t4 + 1) * 128],
                                        xts[e][:],
                                        start=(e == 0), stop=(e == NET - 1))
                                with nc.allow_low_precision(reason="f32r"):
                                    nc.vector.tensor_scalar_add(
                                        qt8[2 * t4][0:64, cols],
                                        ps[0:64, :],
                                        bqk_sb[0:64, t4:t4 + 1])
                                    nc.vector.tensor_scalar_add(
                                        qt8[2 * t4 + 1][64:128, cols],
                                        ps[64:128, :],
                                        bqk_sb[64:128, t4:t4 + 1])
                        elif wi == 1:         # KT (2 heads packed)
                            for t4 in range(4):
                                ps = psp.tile([128, 512], F32, tag="acc",
                                              bufs=2, name="accp")
                                for e in range(NET):
                                    nc.tensor.matmul(
                                        ps[:],
                                        ws[e][:, t4 * 128:(t4 + 1) * 128],
                                        xts[e][:],
                                        start=(e == 0), stop=(e == NET - 1))
                                with nc.allow_low_precision(reason="f32r"):
                                    nc.vector.tensor_scalar_add(
                                        kt_t[t4][:, cols], ps[:],
                                        bqk_sb[:, 4 + t4:5 + t4])
                        else:                 # V (natural layout)
                            for st in range(4):
                                gst = qt * 4 + st
                                ps = psp.tile([128, 512], F32, tag="acc",
                                              bufs=2, name="accp")
                                for e in range(NET):
                                    nc.tensor.matmul(
                                        ps[:],
                                        xts[e][:, st * 128:(st + 1) * 128],
                                        ws[e][:],
                                        start=(e == 0), stop=(e == NET - 1))
                                vt = v_t[gst]
                                v3 = vt[:].rearrange(
                                    "p (h d) -> p h d", h=HL, d=65)
                                with nc.allow_low_precision(reason="f32r"):
                                    nc.vector.tensor_tensor(
                                        out=v3[:, :, 0:64],
                                        in0=ps[:].rearrange(
                                            "p (h d) -> p h d", h=HL, d=64),
                                        in1=bvb_sb[:].rearrange(
                                            "p (h d) -> p h d", h=HL, d=64),
                                        op=add)
                                nc.sync.dma_start(
                                    v3[:, :, 64:65],
                                    onescol.ap().rearrange(
                                        "p (h o) -> p h o", o=1))

            # ---- phase 2 (attention) + phase 3 (out-proj, per q-block) ----
            with (
                tc.tile_pool(name="ph2", bufs=1) as p2,
                tc.tile_pool(name="ph3", bufs=1) as p3,
            ):
                tri_sb = p2.tile([128, 128], F32R, tag="mask", name="tri")
                nc.sync.dma_start(tri_sb[:], maskt.ap())
                bob_sb = p3.tile([128, E], F32, tag="bob", name="bob")
                nc.sync.dma_start(bob_sb[:], bob.ap())
                wos = []
                for ft in range(4):
                    t = p3.tile([128, E], F32R, tag="wo", bufs=4,
                                name=f"wo{ft}")
                    nc.sync.dma_start(t[:], wo.ap()[ft * 128:(ft + 1) * 128, :])
                    wos.append(t)

                partials = [dram.tile([256, E], F32, tag="partial", bufs=8,
                                      name=f"partial{i}") for i in range(8)]
                rs_outs = [dram.tile([128, E], F32, tag="rsout", bufs=8,
                                     name=f"rsout{i}") for i in range(8)]
                outdma_q = []                 # chunks awaiting final out-DMA

                def flush_outdma(upto):
                    while outdma_q and outdma_q[0] <= upto:
                        c = outdma_q.pop(0)
                        nc.sync.dma_start(
                            out.ap()[128 * c:128 * (c + 1), :], rs_outs[c][:])

                pending = [None]              # deferred normalization

                def normalize():
                    po, ctxd, ctx_ps = pending[0]
                    denom = p2.tile([1, 512], F32, tag="denom", bufs=2,
                                    name="denom")
                    nc.vector.tensor_copy(denom[:], ctx_ps[64:65, :])
                    recip = p2.tile([1, 512], F32, tag="recip", bufs=2,
                                    name="recip")
                    nc.vector.reciprocal_approx_fast(
                        out=recip[:], in_=denom[:])
                    recip_b = p2.tile([64, 512], F32, tag="recipb", bufs=2,
                                      name="recipb")
                    nc.gpsimd.partition_broadcast(recip_b[:], recip[:])
                    with nc.allow_low_precision(reason="f32r"):
                        nc.vector.tensor_tensor(
                            out=ctxd[po:po + 64, :],
                            in0=ctx_ps[0:64, :], in1=recip_b[:], op=mult)
                    pending[0] = None

                for qb in range(NQT):
                    q0 = qb * 512
                    nk = 4 * qb + 4           # k-tiles (causal)
                    ctx4 = [p2.tile([128, 512], F32R, tag="ctxt", bufs=6,
                                    name=f"ctxt{qb}_{i}") for i in range(4)]
                    for h in range(HL):
                        t4, po = h >> 1, 64 * (h & 1)
                        ctx_ps = psp.tile([128, 512], F32, tag="ctx", bufs=2,
                                          name="ctxp")
                        attn_tiles = []
                        for p in range(nk // 2):   # k-tile pairs
                            sc_ps = psp.tile([128, 1024], F32, tag="sc",
                                             bufs=2, name="scp")
                            for u in range(2):
                                j = 2 * p + u
                                d = max(0, 128 * j - q0)
                                nc.tensor.matmul(
                                    sc_ps[:, u * 512 + d:(u + 1) * 512],
                                    kt_t[t4][:, j * 128:(j + 1) * 128],
                                    qt8[h][:, q0 + d:q0 + 512],
                                    start=True, stop=True)
                            at = p2.tile([128, 1024], F32R, tag="attn",
                                         bufs=6, name="attn")
                            d0 = max(0, 128 * 2 * p - q0)
                            with nc.allow_low_precision(reason="f32r"):
                                nc.scalar.activation(
                                    at[:, d0:1024], sc_ps[:, d0:1024], Exp,
                                    scale=SC)
                            for u in range(2):
                                j = 2 * p + u
                                d = 128 * j - q0
                                if d >= 0:    # diagonal: tri-mask the band
                                    with nc.allow_low_precision(reason="f32r"):
                                        nc.vector.tensor_tensor(
                                            out=at[:, u * 512 + d:
                                                   u * 512 + d + 128],
                                            in0=at[:, u * 512 + d:
                                                   u * 512 + d + 128],
                                            in1=tri_sb[:], op=mult)
                            attn_tiles.append(at)
                        for p in range(nk // 2):
                            at = attn_tiles[p]
                            for u in range(2):
                                j = 2 * p + u
                                d = max(0, 128 * j - q0)
                                nc.tensor.matmul(
                                    ctx_ps[0:65, d:512],
                                    v_t[j][:, h * 65:(h + 1) * 65],
                                    at[:, u * 512 + d:(u + 1) * 512],
                                    start=(j == 0), stop=(j == nk - 1))
                        if pending[0] is not None:
                            normalize()
                        pending[0] = (po, ctx4[t4], ctx_ps)
                    normalize()

                    # ---- out-proj + reduce-scatter for this q-chunk ----
                    for stl in range(4):
                        ck = 2 * qb + stl // 2
                        for eh in range(2):
                            ps = psp.tile([128, 512], F32, tag="acc", bufs=2,
                                          name="accp3")
                            for ft in range(4):
                                nc.tensor.matmul(
                                    ps[:],
                                    ctx4[ft][:, stl * 128:(stl + 1) * 128],
                                    wos[ft][:, eh * 512:(eh + 1) * 512],
                                    start=(ft == 0), stop=(ft == 3))
                            ob = p3.tile([128, 512], F32, tag="outp", bufs=3,
                                         name="outp")
                            nc.vector.tensor_tensor(
                                out=ob[:], in0=ps[:],
                                in1=bob_sb[:, eh * 512:(eh + 1) * 512], op=add)
                            nc.sync.dma_start(
                                partials[ck][(stl % 2) * 128:
                                             (stl % 2) * 128 + 128,
                                             eh * 512:(eh + 1) * 512], ob[:])
                        if stl % 2 == 1:
                            # final out-DMAs deferred 2 chunks so their
                            # cc-sem wait never heads the sync DMA queue
                            flush_outdma(ck - 2)
                            nc.gpsimd.collective_compute(
                                "ReduceScatter",
                                mybir.AluOpType.add,
                                replica_groups=[[0, 1], [2, 3], [4, 5],
                                                [6, 7]],
                                ins=[partials[ck].opt()],
                                outs=[rs_outs[ck].opt()],
                            )
                            outdma_q.append(ck)
                flush_outdma(7)

    nc.compile()
    return nc


def _in_maps(inputs):
    x = np.asarray(inputs["x"], dtype=np.float32)
    Wq = np.asarray(inputs["Wq"], dtype=np.float32)
    bq = np.asarray(inputs["bq"], dtype=np.float32)
    Wk = np.asarray(inputs["Wk"], dtype=np.float32)
    bk = np.asarray(inputs["bk"], dtype=np.float32)
    Wv = np.asarray(inputs["Wv"], dtype=np.float32)
    bv = np.asarray(inputs["bv"], dtype=np.float32)
    Wo = np.asarray(inputs["Wo"], dtype=np.float32)
    bo = np.asarray(inputs["bo"], dtype=np.float32)

    tri = np.triu(np.ones((128, 128), dtype=np.float32))
    maps = []
    for c in range(N_CORES):
        b, hb = c // 2, c % 2
        hs = slice(hb * HL, (hb + 1) * HL)
        wq_c = np.ascontiguousarray(
            Wq[hs].transpose(1, 0, 2).reshape(E, HL * Dh))
        wk_c = np.ascontiguousarray(
            Wk[hs].transpose(1, 0, 2).reshape(E, HL * Dh))
        wv_c = np.ascontiguousarray(
            Wv[hs].transpose(1, 0, 2).reshape(E, HL * Dh))
        bqk_c = np.concatenate(
            [bq[hs].reshape(4, 128).T, bk[hs].reshape(4, 128).T], axis=1)
        bvb_c = np.broadcast_to(
            bv[hs].reshape(1, HL * Dh), (128, HL * Dh)).copy()
        bob_c = (np.broadcast_to(bo, (128, E)).copy() if hb == 0
                 else np.zeros((128, E), dtype=np.float32))
        maps.append({
            "xT": np.ascontiguousarray(x[b].T),
            "wq": wq_c, "wk": wk_c, "wv": wv_c,
            "wo": np.ascontiguousarray(Wo[hb * 512:(hb + 1) * 512]),
            "bqk": np.ascontiguousarray(bqk_c),
            "bvb": bvb_c, "bob": bob_c,
            "maskt": tri,
            "onescol": np.ones((128, 8), dtype=np.float32),
        })
    return maps


def kernel(**inputs) -> np.ndarray:
    from concourse.bass_utils import run_bass_kernel_spmd

    if "nc" not in _CACHE:
        _CACHE["nc"] = _build()
    nc = _CACHE["nc"]

    maps = _in_maps(inputs)
    res = run_bass_kernel_spmd(nc, maps, core_ids=list(range(N_CORES)),
                               **_CACHE.get("run_kwargs", {}))
    _CACHE["last_results"] = res

    # chunk ck of the pairwise reduce-scatter holds summed rows
    # [256*ck + 128*hb, +128) at out[128*ck : 128*(ck+1)]
    out = np.empty((B, S, E), dtype=np.float32)
    for c in range(N_CORES):
        b, hb = c // 2, c % 2
        o = res.results[c]["out"]
        for ck in range(8):
            out[b, 256 * ck + 128 * hb:256 * ck + 128 * hb + 128, :] = \
                o[128 * ck:128 * (ck + 1)]
    return out
